# revision 34
# baseline (speedup 1.0000x reference)
"""Distributed multi-head attention forward for 8 TRN2 NeuronCores.

Problem: B=2, N=2048, D=768, 12 heads x 64 head-dim, f32.
  qkv = x @ w_qkv + b_qkv ; per-head softmax(q k^T / 8) v ; out proj.

Sharding: core = 4*b + g  (b = batch element, g = query-chunk of 512 rows).
Each core:
  - computes K^T / V for its 512 tokens in NCC=4 128-token chunks, kicking
    a bf16 AllGather per chunk (V carries a per-head ones-column so P@V
    also yields the softmax denominator),
  - Q^T for its 512 rows (overlapping the collectives),
  - attention over key chunks as the gathers land, software-pipelined by
    one chunk with the emission order sc01(c) pv01(c-1) sc23(c) pv23(c-1)
    and scores split across two independent 2-bank psum tiles, so neither
    the PE nor the scalar-engine exp ever head-of-line blocks on the
    other; psum partials are evacuated into f32 SBUF accumulators per
    gather phase (flash-style, no max subtraction needed at this scale),
  - softmax normalization (batched fast-approx reciprocal, DRAM-bounce
    partition broadcast, gpsimd multiply) pipelined per head-quad into the
    last phase,
  - output projection, contraction-split so head-quads 0/1 accumulate
    while head-quad 2 still normalizes.  Host concatenates the row blocks.

All matmul operands are bf16 (f32 psum accumulation; bf16 gathers keep the
rel err ~5e-3 - fp8 gathers were measured at 2.8e-2, over the 2e-2 gate).
Weights arrive host-packed in SBUF layout so their DMAs are a few large
descriptors; the per-token-chunk collective layout is partition-major for
the same reason.
"""

import numpy as np
import ml_dtypes

import concourse.bass as bass
import concourse.tile as tile
from concourse import bacc, mybir
from concourse.bass import ts, ds
from concourse.bass_utils import run_bass_kernel_spmd
from concourse.masks import make_identity

FP = mybir.dt.float32
FR = mybir.dt.float32r
BF = mybir.dt.bfloat16
F8 = mybir.dt.float8e4

P = 128
T = 512            # tokens (query rows) per core
D = 768            # model dim
H = 12             # heads
DH = 64            # head dim
VA = H * (DH + 1)  # 780: v columns + per-head ones column
KEYS = 2048
DC = D // P        # 6 chunks of the contraction dim
TT = T // P        # 4 token tiles per core
NKC = KEYS // P    # 16 key chunks
GROUP = 4          # cores per batch group
SCALE = DH ** -0.5

NCC = 4            # collective chunks (token granularity T/NCC = 128)
TC = T // NCC      # tokens per collective chunk
NJJ = 3            # head quads (4 heads each)

REPLICA_GROUPS = [[0, 1, 2, 3], [4, 5, 6, 7]]

# per-rank collective block: p-major [P, DC*TC (K^T) + VA (V)]
CW = DC * TC + VA  # free elements per partition

_DEBUG = False  # adds intermediate-tensor dumps (debug.py)


def build_nc():
    nc = bacc.Bacc(
        "TRN2",
        target_bir_lowering=False,
        debug=False,
        enable_asserts=False,
        num_devices=8,
    )
    x = nc.dram_tensor("x", [T, D], FP, kind="ExternalInput").ap()
    # host-packed weights: [p, ct, dc, c] / [p, dc, va] / [p, dc, oc]
    wq = nc.dram_tensor("wq", [P, DC * DC * P], BF, kind="ExternalInput").ap()
    wk = nc.dram_tensor("wk", [P, DC * DC * P], BF, kind="ExternalInput").ap()
    wv = nc.dram_tensor("wv", [P, DC * VA], BF, kind="ExternalInput").ap()
    wo = nc.dram_tensor("wo", [P, DC * D], BF, kind="ExternalInput").ap()
    bqk = nc.dram_tensor("bqk", [P, 2 * DC], FP, kind="ExternalInput").ap()
    bv = nc.dram_tensor("bv", [VA], FP, kind="ExternalInput").ap()
    bo = nc.dram_tensor("bo", [D], FP, kind="ExternalInput").ap()
    out = nc.dram_tensor("out", [T, D], FP, kind="ExternalOutput").ap()
    dbg = {}
    if _DEBUG:
        dbg = dict(
            qt=nc.dram_tensor("dbg_qt", [P, DC, T], BF, kind="ExternalOutput").ap(),
            kt=nc.dram_tensor("dbg_kt", [P, NKC, DC, P], BF, kind="ExternalOutput").ap(),
            v=nc.dram_tensor("dbg_v", [P, NKC, VA], BF, kind="ExternalOutput").ap(),
            acc=nc.dram_tensor("dbg_acc", [DH + 1, H, T], FP, kind="ExternalOutput").ap(),
            ot=nc.dram_tensor("dbg_ot", [P, DC, T], BF, kind="ExternalOutput").ap(),
            rec=nc.dram_tensor("dbg_rec", [H, T], FP, kind="ExternalOutput").ap(),
        )

    with tile.TileContext(nc) as tc:
        _build_body(tc, x, wq, wk, wv, bqk, bv, wo, bo, out, dbg)
    nc.compile()
    return nc


def _build_body(tc, x, wq, wk, wv, bqk, bv, wo, bo, out, dbg=()):
    nc = tc.nc
    Add = mybir.AluOpType.add
    Mult = mybir.AluOpType.mult
    Exp = mybir.ActivationFunctionType.Exp

    big = tc.alloc_tile_pool(name="big", bufs=1)
    stream = tc.alloc_tile_pool(name="stream", bufs=2)
    singles = tc.alloc_tile_pool(name="singles", bufs=1)
    psum = tc.alloc_tile_pool(name="psum", bufs=1, space="PSUM")
    dram = tc.alloc_tile_pool(name="dram", bufs=1, space="DRAM")

    # psum: "half" = 1 bank x4 bufs, "quad" = 4 contiguous banks x1 -> 8 banks
    def half(name):
        return psum.tile([P, T], FP, tag="half", bufs=4, name=name)

    def halfq(tag, name):
        return psum.tile([P, 2 * T], FP, tag=tag, bufs=1, name=name)

    # ---- constants ----
    # warm the exp activation table before anything else needs ACT
    warm_in = singles.tile([1, 8], FP)
    nc.vector.memset(warm_in, 0.0)
    warm_out = singles.tile([1, 8], FP)
    nc.scalar.activation(warm_out, warm_in, Exp)

    identity = singles.tile([P, P], FP)
    make_identity(nc, identity)
    ones64 = singles.tile([1, DH], FP)
    nc.vector.memset(ones64, 1.0)
    bqk_sb = singles.tile([P, 2 * DC], FP)
    nc.sync.dma_start(bqk_sb, bqk)
    bq_sb = bqk_sb[:, 0:DC]
    bk_sb = bqk_sb[:, DC : 2 * DC]
    bv_bc = singles.tile([P, VA], FP)
    nc.gpsimd.dma_start(
        out=bv_bc, in_=bass.AP(tensor=bv.tensor, offset=bv.offset, ap=[[0, P], *bv.ap])
    )
    bo_bc = singles.tile([P, D], FP)
    nc.gpsimd.dma_start(
        out=bo_bc, in_=bass.AP(tensor=bo.tensor, offset=bo.offset, ap=[[0, P], *bo.ap])
    )

    # ---- persistent SBUF tensors ----
    xT = big.tile([P, DC, T], BF)          # x^T for this core's tokens
    QT = big.tile([P, DC, T], BF)          # Q^T (biased)
    OT03 = big.tile([P, 4, T], BF)         # normalized attention out (j 0-3)
    OT45 = big.tile([P, 2, T], BF)         # normalized attention out (j 4-5)
    OT_acc = big.tile([DH + 1, H, T], FP)  # flash accumulators (f32)
    KT_all = big.tile([P, NKC, DC, P], BF) # gathered K^T, key-chunk major
    V_all = big.tile([P, NKC, VA], BF)     # gathered V (+ones cols)
    wk_sb = big.tile([P, DC, DC, P], BF)   # [p, ct(out), dc(in), c]
    wq_sb = big.tile([P, DC, DC, P], BF)
    wv_sb = big.tile([P, DC, VA], BF)
    wo_sb = big.tile([P, DC, D], BF)

    # ---- collective bounce buffers (NCC chunked bf16 AllGathers) ----
    # p-major layout: [p, 0:DC*TC] = K^T rows, [p, DC*TC:CW] = V row
    cc_in = [dram.tile([P * CW], BF, name=f"cc_in{c}") for c in range(NCC)]
    cc_out = [dram.tile([GROUP * P * CW], BF, name=f"cc_out{c}") for c in range(NCC)]

    # ---- front: per collective chunk c (128 tokens):
    #      load x tile, transpose, K^T proj, V proj, kick AllGather ----
    # x tile 0 first, then per-ct wk and per-dc wv slices so chunk 0's
    # projections start as early as possible
    x_ts = []
    for c in range(NCC):
        x_t = stream.tile([P, D], FP, tag="xs", bufs=NCC, name="x_t")
        x_ts.append(x_t)
        if c == 0:
            nc.sync.dma_start(x_t, x[ts(c, P), :])
    for ct in range(DC):
        nc.sync.dma_start(wk_sb[:, ct, :, :], wk[:, ds(ct * DC * P, DC * P)])
    for dc in range(DC):
        nc.sync.dma_start(wv_sb[:, dc, :], wv[:, ds(dc * VA, VA)])
    for c in range(1, NCC):
        nc.sync.dma_start(x_ts[c], x[ts(c, P), :])
    for c in range(NCC):
        x_t = x_ts[c]
        for dc in range(DC):
            pt = half("tp")
            nc.tensor.transpose(pt[:P, :P], x_t[:, ts(dc, P)], identity)
            nc.vector.tensor_copy(out=xT[:, dc, ts(c, P)], in_=pt[:P, :P])
        kt_c = stream.tile([P, DC, TC], BF, tag="ktc", bufs=2, name="kt_c")
        for ct in range(DC):
            pk = half("pk")
            for dc in range(DC):
                nc.tensor.matmul(
                    pk[:, :TC], wk_sb[:, ct, dc, :], xT[:, dc, ds(c * TC, TC)],
                    start=(dc == 0), stop=(dc == DC - 1),
                )
            nc.vector.tensor_tensor(
                out=kt_c[:, ct, :], in0=pk[:, :TC],
                in1=bk_sb[:, ct : ct + 1].to_broadcast([P, TC]), op=Add,
            )
        nc.sync.dma_start(
            cc_in[c][0 : P * DC * TC].rearrange("(p t) -> p t", p=P), kt_c
        )
        pv = halfq("sca", "pvproj")
        for dc in range(DC):
            for lo, sz in ((0, T), (T, VA - T)):
                nc.tensor.matmul(
                    pv[:, ds(lo, sz)],
                    xT[:, dc, ts(c, P)],
                    wv_sb[:, dc, ds(lo, sz)],
                    start=(dc == 0), stop=(dc == DC - 1),
                )
        v_stage = stream.tile([P, VA], BF, tag="vs", bufs=2, name="v_stage")
        nc.vector.tensor_tensor(out=v_stage, in0=pv[:, :VA], in1=bv_bc, op=Add)
        nc.sync.dma_start(
            cc_in[c][P * DC * TC : P * CW].rearrange("(p t) -> p t", p=P), v_stage
        )
        nc.gpsimd.collective_compute(
            "AllGather", mybir.AluOpType.bypass,
            replica_groups=REPLICA_GROUPS,
            ins=[cc_in[c].opt()], outs=[cc_out[c].opt()],
        )

    # ---- Q^T projection (overlaps the collectives) ----
    nc.sync.dma_start(wq_sb, wq)
    nc.sync.dma_start(wo_sb, wo)
    for ct in range(DC):
        pq = half("pq")
        for dc in range(DC):
            nc.tensor.matmul(
                pq, wq_sb[:, ct, dc, :], xT[:, dc, :],
                start=(dc == 0), stop=(dc == DC - 1),
            )
        nc.vector.tensor_tensor(
            out=QT[:, ct, :], in0=pq,
            in1=bq_sb[:, ct : ct + 1].to_broadcast([P, T]), op=Add,
        )

    # ---- normalization helpers (emitted per head-quad, pipelined) ----
    # one rec bounce tile per head-quad: a single shared tile would make
    # quad 0's broadcast reads wait on quad 2's reciprocal write
    rec_drams = [dram.tile([4, T], FP, name=f"rec_dram{jj}") for jj in range(NJJ)]

    def norm_den(den4, jj, q):
        h = 4 * jj + q
        nc.sync.dma_start(den4[q : q + 1, :], OT_acc[DH : DH + 1, h, :])

    def norm_quad(jj, den4):
        rec4 = stream.tile([4, T], FP, tag="rec", bufs=2, name="rec4")
        nc.vector.reciprocal_approx_fast(rec4, den4)
        nc.sync.dma_start(rec_drams[jj], rec4)
        for q in range(4):
            h = 4 * jj + q
            j, hl = h // 2, h % 2
            ot_t = OT03 if j < 4 else OT45
            ot_sl = ot_t[ds(hl * DH, DH), j if j < 4 else j - 4, :]
            if jj == NJJ - 1:
                # tail-critical quad: SBUF row copy + ones-matmul broadcast
                # (PE and psum are free here) - 2 hops instead of 4
                rec_q = stream.tile([1, T], FP, tag="recq", bufs=4, name="rec_q")
                nc.sync.dma_start(rec_q, rec4[q : q + 1, :])
                bc_ps = half(f"bc{q}")
                nc.tensor.matmul(
                    bc_ps[:DH, :], ones64, rec_q, start=True, stop=True
                )
                nc.vector.tensor_tensor(
                    out=ot_sl, in0=OT_acc[:DH, h, :], in1=bc_ps[:DH, :], op=Mult,
                )
            else:
                rrow = rec_drams[jj][q, :]
                bc = stream.tile([DH, T], FP, tag="bc", bufs=4, name="bc")
                dma_eng = nc.gpsimd if q % 2 == 0 else nc.sync
                dma_eng.dma_start(
                    out=bc,
                    in_=bass.AP(
                        tensor=rrow.tensor, offset=rrow.offset, ap=[[0, DH], *rrow.ap]
                    ),
                )
                eng = nc.gpsimd if q % 2 == 0 else nc.vector
                eng.tensor_tensor(
                    out=ot_sl, in0=OT_acc[:DH, h, :], in1=bc, op=Mult,
                )

    # ---- attention, pipelined over gather chunks ----
    # key chunk kc = r*TT + c  <->  keys r*T + c*TC ...
    for c in range(NCC):
        for r in range(GROUP):
            kc = r * TT + c
            blk = cc_out[c][r * P * CW : (r + 1) * P * CW]
            nc.sync.dma_start(
                KT_all[:, kc, :, :],
                blk[0 : P * DC * TC].rearrange("(p q t) -> p q t", p=P, q=DC),
            )
            nc.sync.dma_start(
                V_all[:, kc, :],
                blk[P * DC * TC : P * CW].rearrange("(p t) -> p t", p=P),
            )
        kcs = [r * TT + c for r in range(GROUP)]
        for jj in range(NJJ):
            pv_acc = [half(f"pv{jj}_{q}") for q in range(4)]
            n_kc = len(kcs)
            es_pend = None  # (es pair, kc index) awaiting its PV matmuls

            def pv_pair(jp, es_p, ci_p):
                for hl in range(2):
                    q = 2 * jp + hl
                    h = 4 * jj + q
                    nc.tensor.matmul(
                        pv_acc[q][: DH + 1, :],
                        V_all[:, kcs[ci_p], ds(h * (DH + 1), DH + 1)],
                        es_p[jp][:, ds(hl * T, T)],
                        start=(ci_p == 0), stop=(ci_p == n_kc - 1),
                    )

            for ci in range(n_kc + 1):
                kc = kcs[ci] if ci < n_kc else None
                sc = (
                    [halfq("sca", f"sca{jj}"), halfq("scb", f"scb{jj}")]
                    if ci < n_kc else None
                )
                es = [None, None] if ci < n_kc else None
                # interleave: scores pair jp of chunk ci between the two PV
                # pairs of chunk ci-1, so the PE never head-of-line blocks
                # on the scalar-engine exp of the other half
                for jp in range(2):
                    if sc is not None:
                        j = 2 * jj + jp
                        for hl in range(2):
                            q = 2 * jp + hl
                            nc.tensor.matmul(
                                sc[jp][:, ds(hl * T, T)],
                                KT_all[ds(hl * DH, DH), kc, j, :],
                                QT[ds(hl * DH, DH), j, :],
                                start=True, stop=True,
                            )
                    if es_pend is not None:
                        pv_pair(jp, *es_pend)
                if sc is not None:
                    for hf in range(2):
                        es[hf] = stream.tile(
                            [P, 2 * T], BF, tag="es", bufs=6, name="es"
                        )
                        nc.scalar.activation(
                            es[hf], sc[hf], Exp, scale=SCALE
                        )
                es_pend = (es, ci) if es is not None else None
            # evacuate psum partials into the f32 accumulators
            den4 = (
                stream.tile([4, T], FP, tag="den", bufs=2, name="den4")
                if c == NCC - 1 else None
            )
            for q in range(4):
                h = 4 * jj + q
                if c == 0:
                    nc.vector.tensor_copy(
                        out=OT_acc[:, h, :], in_=pv_acc[q][: DH + 1, :]
                    )
                else:
                    nc.vector.tensor_tensor(
                        out=OT_acc[:, h, :], in0=pv_acc[q][: DH + 1, :],
                        in1=OT_acc[:, h, :], op=Add,
                    )
                if den4 is not None:
                    norm_den(den4, jj, q)
            if den4 is not None:
                norm_quad(jj, den4)

    # ---- output projection ----
    # dc 0-3 (head-quads 0/1) accumulate while head-quad 2 still normalizes;
    # all four token tiles in flight: tt0/tt1 on the 2-bank tiles, tt2/tt3
    # on pairs of the freed 1-bank tiles
    po_parts = []  # per tt: list of (psum_ap, lo, sz)
    for tt in range(TT):
        if tt == 0:
            po = halfq("sca", "po")
            po_parts.append([(po[:, 0:T], 0, T), (po[:, ds(T, D - T)], T, D - T)])
        elif tt == 1:
            po = halfq("scb", "po")
            po_parts.append([(po[:, 0:T], 0, T), (po[:, ds(T, D - T)], T, D - T)])
        else:
            pa, pb = half("poa"), half("pob")
            po_parts.append([(pa[:, 0:T], 0, T), (pb[:, 0 : D - T], T, D - T)])

    def po_part(tt, dcs, start, stop):
        for dc in dcs:
            ot_sl = OT03[:, dc, :] if dc < 4 else OT45[:, dc - 4, :]
            for ap, lo, sz in po_parts[tt]:
                nc.tensor.matmul(
                    ap, ot_sl[:, ts(tt, P)], wo_sb[:, dc, ds(lo, sz)],
                    start=start and dc == dcs[0],
                    stop=stop and dc == dcs[-1],
                )

    for tt in range(TT):
        po_part(tt, [0, 1, 2, 3], True, False)
    for tt in range(TT):
        po_part(tt, [4, 5], False, True)
        o_stage = stream.tile([P, D], FP, tag="xs", bufs=NCC, name="o_stage")
        nc.vector.tensor_tensor(
            out=o_stage[:, 0:T], in0=po_parts[tt][0][0], in1=bo_bc[:, 0:T], op=Add
        )
        nc.vector.tensor_tensor(
            out=o_stage[:, ds(T, D - T)], in0=po_parts[tt][1][0],
            in1=bo_bc[:, ds(T, D - T)], op=Add,
        )
        nc.sync.dma_start(out[ts(tt, P), :], o_stage)

    if dbg:
        nc.sync.dma_start(dbg["qt"], QT)
        nc.sync.dma_start(dbg["kt"], KT_all)
        nc.sync.dma_start(dbg["v"], V_all)
        nc.sync.dma_start(dbg["acc"], OT_acc)
        nc.sync.dma_start(dbg["rec"], rec_dram)

    for pool in (dram, psum, singles, stream, big):
        pool.release()


_CACHE = {}


def _get_nc():
    if "nc" not in _CACHE:
        _CACHE["nc"] = build_nc()
    return _CACHE["nc"]


def _prep_inputs(x, w_qkv, b_qkv, w_out, b_out):
    x = np.ascontiguousarray(np.asarray(x, np.float32))
    w_qkv = np.asarray(w_qkv, np.float32)
    b_qkv = np.asarray(b_qkv, np.float32)
    w_out = np.asarray(w_out, np.float32)
    b_out = np.ascontiguousarray(np.asarray(b_out, np.float32))

    bf = ml_dtypes.bfloat16

    def pack_proj(w):  # [768, 768] -> [p, ct, dc, c] flattened
        return np.ascontiguousarray(
            w.reshape(DC, P, DC, P).transpose(1, 2, 0, 3).reshape(P, DC * DC * P)
        ).astype(bf)

    wq = pack_proj(w_qkv[:, 0:768])
    wk = pack_proj(w_qkv[:, 768:1536])
    wv_raw = w_qkv[:, 1536:2304]
    bq = b_qkv[0:768]
    bk = b_qkv[768:1536]
    bv_raw = b_qkv[1536:2304]

    wv_f = np.zeros((D, VA), np.float32)
    bv = np.zeros((VA,), np.float32)
    for h in range(H):
        wv_f[:, h * 65 : h * 65 + 64] = wv_raw[:, h * 64 : (h + 1) * 64]
        bv[h * 65 : h * 65 + 64] = bv_raw[h * 64 : (h + 1) * 64]
        bv[h * 65 + 64] = 1.0
    # [p, dc, va] packing
    wv = np.ascontiguousarray(
        wv_f.reshape(DC, P, VA).transpose(1, 0, 2).reshape(P, DC * VA)
    ).astype(bf)
    wo = np.ascontiguousarray(
        w_out.reshape(DC, P, D).transpose(1, 0, 2).reshape(P, DC * D)
    ).astype(bf)
    # [p, 2*DC] = [bq | bk] column-major by d-chunk
    bqk = np.concatenate(
        [bq.reshape(DC, P).T, bk.reshape(DC, P).T], axis=1
    ).astype(np.float32)
    bqk = np.ascontiguousarray(bqk)

    in_maps = []
    for b in range(2):
        for g in range(GROUP):
            in_maps.append(
                dict(
                    x=np.ascontiguousarray(x[b, g * T : (g + 1) * T]),
                    wq=wq, wk=wk, wv=wv, bqk=bqk, bv=bv,
                    wo=wo, bo=b_out,
                )
            )
    return in_maps


def run_on_hw(x, w_qkv, b_qkv, w_out, b_out, **kwargs):
    in_maps = _prep_inputs(x, w_qkv, b_qkv, w_out, b_out)
    res = run_bass_kernel_spmd(_get_nc(), in_maps, core_ids=list(range(8)), **kwargs)
    full = np.empty((2, 2048, D), np.float32)
    for b in range(2):
        for g in range(GROUP):
            full[b, g * T : (g + 1) * T] = res.results[b * GROUP + g]["out"]
    return full, res


def kernel(x, w_qkv, b_qkv, w_out, b_out):
    full, _ = run_on_hw(x, w_qkv, b_qkv, w_out, b_out)
    return full


# revision 35
# speedup vs baseline: 1.0791x; 1.0791x over previous
"""Distributed multi-head attention forward for 8 TRN2 NeuronCores.

Problem: B=2, N=2048, D=768, 12 heads x 64 head-dim, f32.
  qkv = x @ w_qkv + b_qkv ; per-head softmax(q k^T / 8) v ; out proj.

Sharding: core = 4*b + g  (b = batch element, g = query-chunk of 512 rows).
Each core:
  - computes K^T / V for its 512 tokens in NCC=4 128-token chunks, kicking
    a bf16 AllGather per chunk (V carries a per-head ones-column so P@V
    also yields the softmax denominator),
  - Q^T for its 512 rows (overlapping the collectives),
  - attention over key chunks as the gathers land, software-pipelined by
    one chunk with the emission order sc01(c) pv01(c-1) sc23(c) pv23(c-1)
    and scores split across two independent 2-bank psum tiles, so neither
    the PE nor the scalar-engine exp ever head-of-line blocks on the
    other; psum partials are evacuated into f32 SBUF accumulators per
    gather phase (flash-style, no max subtraction needed at this scale),
  - softmax normalization (batched fast-approx reciprocal, DRAM-bounce
    partition broadcast, gpsimd multiply) pipelined per head-quad into the
    last phase,
  - output projection, contraction-split so head-quads 0/1 accumulate
    while head-quad 2 still normalizes.  Host concatenates the row blocks.

All matmul operands are bf16 (f32 psum accumulation; bf16 gathers keep the
rel err ~5e-3 - fp8 gathers were measured at 2.8e-2, over the 2e-2 gate).
Weights arrive host-packed in SBUF layout so their DMAs are a few large
descriptors; the per-token-chunk collective layout is partition-major for
the same reason.
"""

import numpy as np
import ml_dtypes

import concourse.bass as bass
import concourse.tile as tile
from concourse import bacc, mybir
from concourse.bass import ts, ds
from concourse.bass_utils import run_bass_kernel_spmd
from concourse.masks import make_identity

FP = mybir.dt.float32
FR = mybir.dt.float32r
BF = mybir.dt.bfloat16
F8 = mybir.dt.float8e4

P = 128
T = 512            # tokens (query rows) per core
D = 768            # model dim
H = 12             # heads
DH = 64            # head dim
VA = H * (DH + 1)  # 780: v columns + per-head ones column
KEYS = 2048
DC = D // P        # 6 chunks of the contraction dim
TT = T // P        # 4 token tiles per core
NKC = KEYS // P    # 16 key chunks
GROUP = 4          # cores per batch group
SCALE = DH ** -0.5

NCC = 4            # collective chunks (token granularity T/NCC = 128)
TC = T // NCC      # tokens per collective chunk
NJJ = 3            # head quads (4 heads each)

REPLICA_GROUPS = [[0, 1, 2, 3], [4, 5, 6, 7]]

# per-rank collective block: p-major [P, DC*TC (K^T) + VA (V)]
CW = DC * TC + VA  # free elements per partition

_DEBUG = False  # adds intermediate-tensor dumps (debug.py)


def build_nc():
    nc = bacc.Bacc(
        "TRN2",
        target_bir_lowering=False,
        debug=False,
        enable_asserts=False,
        num_devices=8,
    )
    x = nc.dram_tensor("x", [T, D], FP, kind="ExternalInput").ap()
    # host-packed weights: [p, ct, dc, c] / [p, dc, va] / [p, dc, oc]
    wq = nc.dram_tensor("wq", [P, DC * DC * P], BF, kind="ExternalInput").ap()
    wk = nc.dram_tensor("wk", [P, DC * DC * P], BF, kind="ExternalInput").ap()
    wv = nc.dram_tensor("wv", [P, DC * VA], BF, kind="ExternalInput").ap()
    wo = nc.dram_tensor("wo", [P, DC * D], BF, kind="ExternalInput").ap()
    bqk = nc.dram_tensor("bqk", [P, 2 * DC], FP, kind="ExternalInput").ap()
    bv = nc.dram_tensor("bv", [VA], FP, kind="ExternalInput").ap()
    bo = nc.dram_tensor("bo", [D], FP, kind="ExternalInput").ap()
    out = nc.dram_tensor("out", [T, D], FP, kind="ExternalOutput").ap()
    dbg = {}
    if _DEBUG:
        dbg = dict(
            qt=nc.dram_tensor("dbg_qt", [P, DC, T], BF, kind="ExternalOutput").ap(),
            kt=nc.dram_tensor("dbg_kt", [P, NKC, DC, P], BF, kind="ExternalOutput").ap(),
            v=nc.dram_tensor("dbg_v", [P, NKC, VA], BF, kind="ExternalOutput").ap(),
            acc=nc.dram_tensor("dbg_acc", [DH + 1, H, T], FP, kind="ExternalOutput").ap(),
            ot=nc.dram_tensor("dbg_ot", [P, DC, T], BF, kind="ExternalOutput").ap(),
            rec=nc.dram_tensor("dbg_rec", [H, T], FP, kind="ExternalOutput").ap(),
        )

    with tile.TileContext(nc) as tc:
        _build_body(tc, x, wq, wk, wv, bqk, bv, wo, bo, out, dbg)
    nc.compile()
    return nc


def _build_body(tc, x, wq, wk, wv, bqk, bv, wo, bo, out, dbg=()):
    nc = tc.nc
    Add = mybir.AluOpType.add
    Mult = mybir.AluOpType.mult
    Exp = mybir.ActivationFunctionType.Exp

    big = tc.alloc_tile_pool(name="big", bufs=1)
    stream = tc.alloc_tile_pool(name="stream", bufs=2)
    singles = tc.alloc_tile_pool(name="singles", bufs=1)
    psum = tc.alloc_tile_pool(name="psum", bufs=1, space="PSUM")

    # psum: "half" = 1 bank x4 bufs, "quad" = 4 contiguous banks x1 -> 8 banks
    def half(name):
        return psum.tile([P, T], FP, tag="half", bufs=4, name=name)

    def halfq(tag, name):
        return psum.tile([P, 2 * T], FP, tag=tag, bufs=1, name=name)

    # ---- constants ----
    # warm the exp activation table before anything else needs ACT
    warm_in = singles.tile([1, 8], FP)
    nc.vector.memset(warm_in, 0.0)
    warm_out = singles.tile([1, 8], FP)
    nc.scalar.activation(warm_out, warm_in, Exp)

    identity = singles.tile([P, P], FP)
    make_identity(nc, identity)
    ones64 = singles.tile([1, DH], FP)
    nc.vector.memset(ones64, 1.0)
    bqk_sb = singles.tile([P, 2 * DC], FP)
    nc.sync.dma_start(bqk_sb, bqk)
    bq_sb = bqk_sb[:, 0:DC]
    bk_sb = bqk_sb[:, DC : 2 * DC]
    bv_bc = singles.tile([P, VA], FP)
    nc.gpsimd.dma_start(
        out=bv_bc, in_=bass.AP(tensor=bv.tensor, offset=bv.offset, ap=[[0, P], *bv.ap])
    )
    bo_bc = singles.tile([P, D], FP)
    nc.gpsimd.dma_start(
        out=bo_bc, in_=bass.AP(tensor=bo.tensor, offset=bo.offset, ap=[[0, P], *bo.ap])
    )

    # ---- persistent SBUF tensors ----
    xT = big.tile([P, DC, T], BF)          # x^T for this core's tokens
    QT = big.tile([P, DC, T], BF)          # Q^T (biased)
    OT03 = big.tile([P, 4, T], BF)         # normalized attention out (j 0-3)
    OT45 = big.tile([P, 2, T], BF)         # normalized attention out (j 4-5)
    OT_acc = big.tile([DH + 1, H, T], FP)  # flash accumulators (f32)
    KT_all = big.tile([P, NKC, DC, P], BF) # gathered K^T, key-chunk major
    V_all = big.tile([P, NKC, VA], BF)     # gathered V (+ones cols)
    wk_sb = big.tile([P, DC, DC, P], BF)   # [p, ct(out), dc(in), c]
    wq_sb = big.tile([P, DC, DC, P], BF)
    wv_sb = big.tile([P, DC, VA], BF)
    wo_sb = big.tile([P, DC, D], BF)

    # ---- collective bounce buffers (NCC chunked bf16 AllGathers) ----
    # p-major layout: [p, 0:DC*TC] = K^T rows, [p, DC*TC:CW] = V row
    cc_in = [
        nc.dram_tensor(f"cc_in{c}", [P * CW], BF, kind="Internal").ap()
        for c in range(NCC)
    ]
    cc_out = [
        nc.dram_tensor(f"cc_out{c}", [GROUP * P * CW], BF, kind="Internal").ap()
        for c in range(NCC)
    ]

    # ---- front: per collective chunk c (128 tokens):
    #      load x tile, transpose, K^T proj, V proj, kick AllGather ----
    # x tile 0 first, then per-ct wk and per-dc wv slices so chunk 0's
    # projections start as early as possible
    x_ts = []
    for c in range(NCC):
        x_t = stream.tile([P, D], FP, tag="xs", bufs=NCC, name="x_t")
        x_ts.append(x_t)
        if c == 0:
            nc.sync.dma_start(x_t, x[ts(c, P), :])
    for ct in range(DC):
        nc.sync.dma_start(wk_sb[:, ct, :, :], wk[:, ds(ct * DC * P, DC * P)])
    for dc in range(DC):
        nc.sync.dma_start(wv_sb[:, dc, :], wv[:, ds(dc * VA, VA)])
    for c in range(1, NCC):
        nc.sync.dma_start(x_ts[c], x[ts(c, P), :])
    for c in range(NCC):
        x_t = x_ts[c]
        for dc in range(DC):
            pt = half("tp")
            nc.tensor.transpose(pt[:P, :P], x_t[:, ts(dc, P)], identity)
            nc.vector.tensor_copy(out=xT[:, dc, ts(c, P)], in_=pt[:P, :P])
        kt_c = stream.tile([P, DC, TC], BF, tag="ktc", bufs=2, name="kt_c")
        for ct in range(DC):
            pk = half("pk")
            for dc in range(DC):
                nc.tensor.matmul(
                    pk[:, :TC], wk_sb[:, ct, dc, :], xT[:, dc, ds(c * TC, TC)],
                    start=(dc == 0), stop=(dc == DC - 1),
                )
            nc.vector.tensor_tensor(
                out=kt_c[:, ct, :], in0=pk[:, :TC],
                in1=bk_sb[:, ct : ct + 1].to_broadcast([P, TC]), op=Add,
            )
        nc.sync.dma_start(
            cc_in[c][0 : P * DC * TC].rearrange("(p t) -> p t", p=P), kt_c
        )
        pv = halfq("sca", "pvproj")
        for dc in range(DC):
            for lo, sz in ((0, T), (T, VA - T)):
                nc.tensor.matmul(
                    pv[:, ds(lo, sz)],
                    xT[:, dc, ts(c, P)],
                    wv_sb[:, dc, ds(lo, sz)],
                    start=(dc == 0), stop=(dc == DC - 1),
                )
        v_stage = stream.tile([P, VA], BF, tag="vs", bufs=2, name="v_stage")
        nc.vector.tensor_tensor(out=v_stage, in0=pv[:, :VA], in1=bv_bc, op=Add)
        nc.sync.dma_start(
            cc_in[c][P * DC * TC : P * CW].rearrange("(p t) -> p t", p=P), v_stage
        )
        nc.gpsimd.collective_compute(
            "AllGather", mybir.AluOpType.bypass,
            replica_groups=REPLICA_GROUPS,
            ins=[cc_in[c].opt()], outs=[cc_out[c].opt()],
        )

    # ---- Q^T projection (overlaps the collectives) ----
    nc.sync.dma_start(wq_sb, wq)
    nc.sync.dma_start(wo_sb, wo)
    for ct in range(DC):
        pq = half("pq")
        for dc in range(DC):
            nc.tensor.matmul(
                pq, wq_sb[:, ct, dc, :], xT[:, dc, :],
                start=(dc == 0), stop=(dc == DC - 1),
            )
        nc.vector.tensor_tensor(
            out=QT[:, ct, :], in0=pq,
            in1=bq_sb[:, ct : ct + 1].to_broadcast([P, T]), op=Add,
        )

    # ---- normalization helpers (emitted per head-quad, pipelined) ----
    # one rec bounce tile per head-quad: a single shared tile would make
    # quad 0's broadcast reads wait on quad 2's reciprocal write
    rec_drams = [
        nc.dram_tensor(f"rec_dram{jj}", [4, T], FP, kind="Internal").ap()
        for jj in range(NJJ)
    ]

    def norm_den(den4, jj, q):
        h = 4 * jj + q
        nc.sync.dma_start(den4[q : q + 1, :], OT_acc[DH : DH + 1, h, :])

    def norm_quad(jj, den4):
        rec4 = stream.tile([4, T], FP, tag="rec", bufs=2, name="rec4")
        nc.vector.reciprocal_approx_fast(rec4, den4)
        nc.sync.dma_start(rec_drams[jj], rec4)
        for q in range(4):
            h = 4 * jj + q
            j, hl = h // 2, h % 2
            ot_t = OT03 if j < 4 else OT45
            ot_sl = ot_t[ds(hl * DH, DH), j if j < 4 else j - 4, :]
            if jj == NJJ - 1:
                # tail-critical quad: SBUF row copy + ones-matmul broadcast
                # (PE and psum are free here) - 2 hops instead of 4
                rec_q = stream.tile([1, T], FP, tag="recq", bufs=4, name="rec_q")
                nc.sync.dma_start(rec_q, rec4[q : q + 1, :])
                bc_ps = half(f"bc{q}")
                nc.tensor.matmul(
                    bc_ps[:DH, :], ones64, rec_q, start=True, stop=True
                )
                nc.vector.tensor_tensor(
                    out=ot_sl, in0=OT_acc[:DH, h, :], in1=bc_ps[:DH, :], op=Mult,
                )
            else:
                rrow = rec_drams[jj][q, :]
                bc = stream.tile([DH, T], FP, tag="bc", bufs=4, name="bc")
                dma_eng = nc.gpsimd if q % 2 == 0 else nc.sync
                dma_eng.dma_start(
                    out=bc,
                    in_=bass.AP(
                        tensor=rrow.tensor, offset=rrow.offset, ap=[[0, DH], *rrow.ap]
                    ),
                )
                eng = nc.gpsimd if q % 2 == 0 else nc.vector
                eng.tensor_tensor(
                    out=ot_sl, in0=OT_acc[:DH, h, :], in1=bc, op=Mult,
                )

    # ---- attention, pipelined over gather chunks ----
    # key chunk kc = r*TT + c  <->  keys r*T + c*TC ...
    for c in range(NCC):
        for r in range(GROUP):
            kc = r * TT + c
            blk = cc_out[c][r * P * CW : (r + 1) * P * CW]
            nc.sync.dma_start(
                KT_all[:, kc, :, :],
                blk[0 : P * DC * TC].rearrange("(p q t) -> p q t", p=P, q=DC),
            )
            nc.sync.dma_start(
                V_all[:, kc, :],
                blk[P * DC * TC : P * CW].rearrange("(p t) -> p t", p=P),
            )
        kcs = [r * TT + c for r in range(GROUP)]
        for jj in range(NJJ):
            pv_acc = [half(f"pv{jj}_{q}") for q in range(4)]
            n_kc = len(kcs)
            es_pend = None  # (es pair, kc index) awaiting its PV matmuls

            def pv_pair(jp, es_p, ci_p):
                for hl in range(2):
                    q = 2 * jp + hl
                    h = 4 * jj + q
                    nc.tensor.matmul(
                        pv_acc[q][: DH + 1, :],
                        V_all[:, kcs[ci_p], ds(h * (DH + 1), DH + 1)],
                        es_p[jp][:, ds(hl * T, T)],
                        start=(ci_p == 0), stop=(ci_p == n_kc - 1),
                    )

            for ci in range(n_kc + 1):
                kc = kcs[ci] if ci < n_kc else None
                sc = (
                    [halfq("sca", f"sca{jj}"), halfq("scb", f"scb{jj}")]
                    if ci < n_kc else None
                )
                es = [None, None] if ci < n_kc else None
                # interleave: scores pair jp of chunk ci between the two PV
                # pairs of chunk ci-1, so the PE never head-of-line blocks
                # on the scalar-engine exp of the other half
                for jp in range(2):
                    if sc is not None:
                        j = 2 * jj + jp
                        for hl in range(2):
                            q = 2 * jp + hl
                            nc.tensor.matmul(
                                sc[jp][:, ds(hl * T, T)],
                                KT_all[ds(hl * DH, DH), kc, j, :],
                                QT[ds(hl * DH, DH), j, :],
                                start=True, stop=True,
                            )
                    if es_pend is not None:
                        pv_pair(jp, *es_pend)
                if sc is not None:
                    for hf in range(2):
                        es[hf] = stream.tile(
                            [P, 2 * T], BF, tag="es", bufs=6, name="es"
                        )
                        nc.scalar.activation(
                            es[hf], sc[hf], Exp, scale=SCALE
                        )
                es_pend = (es, ci) if es is not None else None
            # evacuate psum partials into the f32 accumulators
            den4 = (
                stream.tile([4, T], FP, tag="den", bufs=2, name="den4")
                if c == NCC - 1 else None
            )
            for q in range(4):
                h = 4 * jj + q
                if c == 0:
                    nc.vector.tensor_copy(
                        out=OT_acc[:, h, :], in_=pv_acc[q][: DH + 1, :]
                    )
                else:
                    nc.vector.tensor_tensor(
                        out=OT_acc[:, h, :], in0=pv_acc[q][: DH + 1, :],
                        in1=OT_acc[:, h, :], op=Add,
                    )
                if den4 is not None:
                    norm_den(den4, jj, q)
            if den4 is not None:
                norm_quad(jj, den4)

    # ---- output projection ----
    # dc 0-3 (head-quads 0/1) accumulate while head-quad 2 still normalizes;
    # all four token tiles in flight: tt0/tt1 on the 2-bank tiles, tt2/tt3
    # on pairs of the freed 1-bank tiles
    po_parts = []  # per tt: list of (psum_ap, lo, sz)
    for tt in range(TT):
        if tt == 0:
            po = halfq("sca", "po")
            po_parts.append([(po[:, 0:T], 0, T), (po[:, ds(T, D - T)], T, D - T)])
        elif tt == 1:
            po = halfq("scb", "po")
            po_parts.append([(po[:, 0:T], 0, T), (po[:, ds(T, D - T)], T, D - T)])
        else:
            pa, pb = half("poa"), half("pob")
            po_parts.append([(pa[:, 0:T], 0, T), (pb[:, 0 : D - T], T, D - T)])

    def po_part(tt, dcs, start, stop):
        for dc in dcs:
            ot_sl = OT03[:, dc, :] if dc < 4 else OT45[:, dc - 4, :]
            for ap, lo, sz in po_parts[tt]:
                nc.tensor.matmul(
                    ap, ot_sl[:, ts(tt, P)], wo_sb[:, dc, ds(lo, sz)],
                    start=start and dc == dcs[0],
                    stop=stop and dc == dcs[-1],
                )

    for tt in range(TT):
        po_part(tt, [0, 1, 2, 3], True, False)
    for tt in range(TT):
        po_part(tt, [4, 5], False, True)
        o_stage = stream.tile([P, D], FP, tag="xs", bufs=NCC, name="o_stage")
        nc.vector.tensor_tensor(
            out=o_stage[:, 0:T], in0=po_parts[tt][0][0], in1=bo_bc[:, 0:T], op=Add
        )
        nc.vector.tensor_tensor(
            out=o_stage[:, ds(T, D - T)], in0=po_parts[tt][1][0],
            in1=bo_bc[:, ds(T, D - T)], op=Add,
        )
        nc.sync.dma_start(out[ts(tt, P), :], o_stage)

    if dbg:
        nc.sync.dma_start(dbg["qt"], QT)
        nc.sync.dma_start(dbg["kt"], KT_all)
        nc.sync.dma_start(dbg["v"], V_all)
        nc.sync.dma_start(dbg["acc"], OT_acc)
        nc.sync.dma_start(dbg["rec"], rec_dram)

    for pool in (psum, singles, stream, big):
        pool.release()


_CACHE = {}


def _get_nc():
    if "nc" not in _CACHE:
        _CACHE["nc"] = build_nc()
    return _CACHE["nc"]


def _prep_inputs(x, w_qkv, b_qkv, w_out, b_out):
    x = np.ascontiguousarray(np.asarray(x, np.float32))
    w_qkv = np.asarray(w_qkv, np.float32)
    b_qkv = np.asarray(b_qkv, np.float32)
    w_out = np.asarray(w_out, np.float32)
    b_out = np.ascontiguousarray(np.asarray(b_out, np.float32))

    bf = ml_dtypes.bfloat16

    def pack_proj(w):  # [768, 768] -> [p, ct, dc, c] flattened
        return np.ascontiguousarray(
            w.reshape(DC, P, DC, P).transpose(1, 2, 0, 3).reshape(P, DC * DC * P)
        ).astype(bf)

    wq = pack_proj(w_qkv[:, 0:768])
    wk = pack_proj(w_qkv[:, 768:1536])
    wv_raw = w_qkv[:, 1536:2304]
    bq = b_qkv[0:768]
    bk = b_qkv[768:1536]
    bv_raw = b_qkv[1536:2304]

    wv_f = np.zeros((D, VA), np.float32)
    bv = np.zeros((VA,), np.float32)
    for h in range(H):
        wv_f[:, h * 65 : h * 65 + 64] = wv_raw[:, h * 64 : (h + 1) * 64]
        bv[h * 65 : h * 65 + 64] = bv_raw[h * 64 : (h + 1) * 64]
        bv[h * 65 + 64] = 1.0
    # [p, dc, va] packing
    wv = np.ascontiguousarray(
        wv_f.reshape(DC, P, VA).transpose(1, 0, 2).reshape(P, DC * VA)
    ).astype(bf)
    wo = np.ascontiguousarray(
        w_out.reshape(DC, P, D).transpose(1, 0, 2).reshape(P, DC * D)
    ).astype(bf)
    # [p, 2*DC] = [bq | bk] column-major by d-chunk
    bqk = np.concatenate(
        [bq.reshape(DC, P).T, bk.reshape(DC, P).T], axis=1
    ).astype(np.float32)
    bqk = np.ascontiguousarray(bqk)

    in_maps = []
    for b in range(2):
        for g in range(GROUP):
            in_maps.append(
                dict(
                    x=np.ascontiguousarray(x[b, g * T : (g + 1) * T]),
                    wq=wq, wk=wk, wv=wv, bqk=bqk, bv=bv,
                    wo=wo, bo=b_out,
                )
            )
    return in_maps


def run_on_hw(x, w_qkv, b_qkv, w_out, b_out, **kwargs):
    in_maps = _prep_inputs(x, w_qkv, b_qkv, w_out, b_out)
    res = run_bass_kernel_spmd(_get_nc(), in_maps, core_ids=list(range(8)), **kwargs)
    full = np.empty((2, 2048, D), np.float32)
    for b in range(2):
        for g in range(GROUP):
            full[b, g * T : (g + 1) * T] = res.results[b * GROUP + g]["out"]
    return full, res


def kernel(x, w_qkv, b_qkv, w_out, b_out):
    full, _ = run_on_hw(x, w_qkv, b_qkv, w_out, b_out)
    return full
